# revision 1
# baseline (speedup 1.0000x reference)
"""Trainium2 Bass kernel for nn_Head_84043920048318 (sparse_attention).

Reference computation (per batch b):
    q = x @ Wq; k = x @ Wk; v = x @ Wv           [T, HS]
    wei = (q @ k.T) * C**-0.5                    [T, T]
    for s:  P = softmax(wei * adjacent[b, s], axis=-1);  out[b, s] = P @ v

Sharding: data-parallel over B across 8 NeuronCores (4 batches each);
projection weights replicated.

Per-core dataflow (all shapes hardcoded):
  - x loaded naturally, transposed on PE -> xT [c, t]
  - qT = Wq.T @ x.T, kT likewise (PE), v natural [u, d] with an appended
    ones-column (for the softmax denominator)
  - wei natural [t, u] from lhsT=qT, rhs=kT
  - per (b, s): adjacent loaded naturally, DVE multiply wei*adj, PE
    transposes the product into PSUM, ACT computes exp(scale*x) into bf16
    SBUF (transposed layout), PE computes P^T-stationary matmuls against
    [v | 1] giving PV and the row sums, DVE normalizes, DMA stores.

exp without max-subtraction is safe: |scale * wei * adj| <~ 8, well within
fp32 exp range, and matches softmax exactly up to rounding.
"""

import numpy as np

B, S, T, C, HS = 32, 8, 512, 128, 128
NCORES = 8
BPC = B // NCORES  # batches per core
TB = T // 128  # 4 row blocks
UB = T // 128  # 4 contraction blocks
SCALE = float(C) ** -0.5

_CACHED = None


def _build_module():
    import concourse.bacc as bacc
    import concourse.mybir as mybir
    from concourse import tile
    from concourse.masks import make_identity

    f32 = mybir.dt.float32
    bf16 = mybir.dt.bfloat16

    nc = bacc.Bacc("TRN2", target_bir_lowering=False, debug=False, num_devices=1)

    x_d = nc.dram_tensor("x", [BPC, T, C], f32, kind="ExternalInput").ap()
    adj_d = nc.dram_tensor("adjacent", [BPC, S, T, T], f32, kind="ExternalInput").ap()
    wq_d = nc.dram_tensor("Wq", [C, HS], f32, kind="ExternalInput").ap()
    wk_d = nc.dram_tensor("Wk", [C, HS], f32, kind="ExternalInput").ap()
    wv_d = nc.dram_tensor("Wv", [C, HS], f32, kind="ExternalInput").ap()
    out_d = nc.dram_tensor("out", [BPC, S, T, HS], f32, kind="ExternalOutput").ap()

    with tile.TileContext(nc) as tc:
        with (
            tc.tile_pool(name="consts", bufs=1) as consts,
            tc.tile_pool(name="bpool", bufs=2) as bpool,
            tc.tile_pool(name="spool", bufs=3) as spool,
            tc.tile_pool(name="tiny", bufs=8) as tiny,
            tc.tile_pool(name="pbig", bufs=1, space="PSUM") as pbig,
            tc.tile_pool(name="psmall", bufs=4, space="PSUM") as psmall,
        ):
            ident = consts.tile([128, 128], f32)
            make_identity(nc, ident)
            wq_sb = consts.tile([C, HS], f32, tag="wq")
            wk_sb = consts.tile([C, HS], f32, tag="wk")
            wv_sb = consts.tile([C, HS], f32, tag="wv")
            nc.sync.dma_start(wq_sb[:], wq_d)
            nc.sync.dma_start(wk_sb[:], wk_d)
            nc.sync.dma_start(wv_sb[:], wv_d)

            for b in range(BPC):
                # ---- load x[b], build xT [c, t] via PE transpose ----
                xb = bpool.tile([128, TB, C], f32, tag="xb")
                nc.sync.dma_start(xb[:], x_d[b].rearrange("(n p) c -> p n c", p=128))
                xT_ps = psmall.tile([C, T], f32, tag="ps")
                for tb in range(TB):
                    nc.tensor.transpose(
                        xT_ps[:, tb * 128 : (tb + 1) * 128], xb[:, tb, :], ident[:]
                    )
                xT = bpool.tile([C, T], f32, tag="xT")
                nc.vector.tensor_copy(xT[:], xT_ps[:])

                # ---- projections: qT/kT [h, t] ----
                qT_ps = psmall.tile([HS, T], f32, tag="ps")
                nc.tensor.matmul(qT_ps[:], wq_sb[:], xT[:])
                qT = bpool.tile([HS, T], f32, tag="qT")
                nc.vector.tensor_copy(qT[:], qT_ps[:])

                kT_ps = psmall.tile([HS, T], f32, tag="ps")
                nc.tensor.matmul(kT_ps[:], wk_sb[:], xT[:])
                kT = bpool.tile([HS, T], f32, tag="kT")
                nc.vector.tensor_copy(kT[:], kT_ps[:])

                # ---- v natural [u, d] + ones column, bf16 ----
                vp = bpool.tile([128, UB, HS + 1], bf16, tag="vp")
                for ub in range(UB):
                    v_ps = psmall.tile([128, HS], f32, tag="ps")
                    nc.tensor.matmul(
                        v_ps[:], xT[:, ub * 128 : (ub + 1) * 128], wv_sb[:]
                    )
                    nc.vector.tensor_copy(vp[:, ub, 0:HS], v_ps[:])
                nc.vector.memset(vp[:, :, HS : HS + 1], 1.0)

                # ---- QK: wei natural [t, (tb, u)] fp32 ----
                wei = bpool.tile([128, TB, T], f32, tag="wei")
                for tb in range(TB):
                    wei_ps = psmall.tile([128, T], f32, tag="ps")
                    nc.tensor.matmul(
                        wei_ps[:], qT[:, tb * 128 : (tb + 1) * 128], kT[:]
                    )
                    nc.scalar.copy(wei[:, tb, :], wei_ps[:])

                outb = bpool.tile([128, S, TB, HS], f32, tag="outb")
                for s in range(S):
                    adj = spool.tile([128, TB, T], f32, tag="adj")
                    nc.sync.dma_start(
                        adj[:], adj_d[b, s].rearrange("(n p) u -> p n u", p=128)
                    )
                    prod = spool.tile([128, TB, T], f32, tag="prod")
                    nc.vector.tensor_mul(prod[:], adj[:], wei[:])

                    prodT_ps = pbig.tile([128, UB, T], f32)
                    for ub in range(UB):
                        for tb in range(TB):
                            nc.tensor.transpose(
                                prodT_ps[:, ub, tb * 128 : (tb + 1) * 128],
                                prod[:, tb, ub * 128 : (ub + 1) * 128],
                                ident[:],
                            )
                    pt = spool.tile([128, UB, T], bf16, tag="pt")
                    nc.scalar.activation(
                        pt[:],
                        prodT_ps[:],
                        mybir.ActivationFunctionType.Exp,
                        scale=SCALE,
                    )

                    for tb in range(TB):
                        av_ps = psmall.tile([128, HS + 1], f32, tag="ps")
                        for ub in range(UB):
                            nc.tensor.matmul(
                                av_ps[:],
                                pt[:, ub, tb * 128 : (tb + 1) * 128],
                                vp[:, ub, :],
                                start=(ub == 0),
                                stop=(ub == UB - 1),
                            )
                        rcp = tiny.tile([128, 1], f32, tag="rcp")
                        nc.vector.reciprocal(rcp[:], av_ps[:, HS : HS + 1])
                        nc.vector.tensor_scalar_mul(
                            outb[:, s, tb, :], av_ps[:, 0:HS], rcp[:]
                        )

                nc.sync.dma_start(
                    out_d[b].rearrange("s (n p) d -> p s n d", p=128), outb[:]
                )

    nc.compile()
    return nc


def _get_module():
    global _CACHED
    if _CACHED is None:
        _CACHED = _build_module()
    return _CACHED


def run_on_hw(in_maps, trace=False, trace_kwargs=None):
    """Run the compiled module on the 8 NeuronCores. Returns BassKernelResults."""
    from concourse.bass_utils import run_bass_kernel_spmd
    from concourse.bass_interp import get_hw_module

    nc = _get_module()
    old_m = nc.m
    nc.m = get_hw_module(nc.m)
    try:
        return run_bass_kernel_spmd(
            nc,
            in_maps,
            core_ids=list(range(NCORES)),
            trace=trace,
            **(trace_kwargs or {}),
        )
    finally:
        nc.m = old_m


def make_in_maps(x, adjacent, Wq, Wk, Wv):
    x = np.ascontiguousarray(x, dtype=np.float32)
    adjacent = np.ascontiguousarray(adjacent, dtype=np.float32)
    Wq = np.ascontiguousarray(Wq, dtype=np.float32)
    Wk = np.ascontiguousarray(Wk, dtype=np.float32)
    Wv = np.ascontiguousarray(Wv, dtype=np.float32)
    return [
        {
            "x": x[c * BPC : (c + 1) * BPC],
            "adjacent": adjacent[c * BPC : (c + 1) * BPC],
            "Wq": Wq,
            "Wk": Wk,
            "Wv": Wv,
        }
        for c in range(NCORES)
    ]


def kernel(**inputs) -> np.ndarray:
    in_maps = make_in_maps(
        inputs["x"], inputs["adjacent"], inputs["Wq"], inputs["Wk"], inputs["Wv"]
    )
    res = run_on_hw(in_maps)
    return np.concatenate([res.results[c]["out"] for c in range(NCORES)], axis=0)
